# revision 27
# baseline (speedup 1.0000x reference)
"""Trainium2 Bass kernel for cosine linear-attention (nn_Attention).

Data-parallel over batch N=16 across 8 NeuronCores (2 batches/core,
weights replicated, no collectives). Per core:

  q = l2norm(x @ Wq.T), k = l2norm(x @ Wk.T), v = (x @ Wv.T) * C^-sigmoid(nc)
  out = (q @ (k^T v per head)) @ Wo.T

Compute runs in bf16 on the TensorEngine, f32 PSUM accumulation.

v5 layout strategy: the host hands the device value-identical f32
inputs re-laid-out so the contraction dim is partition-major
(transpose/reshape only -- every arithmetic op including the f32->bf16
rounding runs on device).  Each operand chunk is then ONE SWDGE cast
DMA (f32 HBM -> bf16 SBUF, both sides contiguous) straight into the
final operand tile: no DRAM scratch, no X-bar transposes (whose
hardware serialization against other DMA killed two earlier versions),
no staging SBUF, no engine casts, and no PE input transposes.  SWDGE
spreads across all 16 DMA engines (~250-350 GB/s aggregate vs ~90
GB/s per HWDGE queue).  Emission order on the gpsimd queue encodes
priority: x0 tile 0, then Wk in m-halves (the first K chain needs only
m 0:512), then the rest.  The kv block-diagonal is extracted with
strided DVE copies into a pre-zeroed bf16 tile.  Batch 1's K
projections are emitted between batch 0's Q phase and its
attention/output phases, and batch 0's last two output tiles after
batch 1's Q phase, so the PE never idles across phase seams.  Output
DMAs ride the scalar HWDGE queue.
"""

import sys

for _p in ("/opt/trn_rl_repo",):
    if _p not in sys.path:
        sys.path.append(_p)

import numpy as np
from contextlib import ExitStack

import concourse.bass as bass
import concourse.tile as tile
from concourse import bacc, mybir
from concourse.masks import make_identity
from concourse.bass_utils import run_bass_kernel_spmd

F32 = mybir.dt.float32
BF16 = mybir.dt.bfloat16

N_CORES = 8
N, C, D = 16, 1024, 1024
H, HD = 16, 64
B = N // N_CORES          # batches per core
P = 128
KC = D // P               # contraction chunks (8)
CT = C // P               # c tiles per batch (8)
MC = D // 512             # 512-wide m chunks (2)
MQ = 4                    # 256-wide m quarters for weight loads
HP = H // 2               # head pairs (8)
LN_C = float(np.log(C))


def build_graph():
    nc = bacc.Bacc("TRN2", target_bir_lowering=False, debug=False,
                   num_devices=N_CORES)
    # x^T per batch, blocked per c-slice: xT[n, ct, p, kc, j] =
    # x[n, ct*128+j, kc*128+p]  (value-identical transpose, host-prepped)
    xT_ext = nc.declare_dram_parameter("xT", [B, CT, P, KC, P], F32,
                                       isOutput=False)
    # W^T in device tile layout: wT[p, kc, m] = W[m, kc*128+p]
    wT_ext = {
        w: nc.declare_dram_parameter(f"{w}T", [P, KC, D], F32,
                                     isOutput=False)
        for w in ("Wq", "Wk", "Wv", "Wo")
    }
    ncst_ext = nc.declare_dram_parameter("norm_const", [1, H, 1, 1], F32,
                                         isOutput=False)
    out_ext = nc.declare_dram_parameter("out", [B, C, D], F32, isOutput=True)

    with tile.TileContext(nc) as tc, ExitStack() as ctx:
        singles = ctx.enter_context(tc.tile_pool(name="singles", bufs=1))
        wt_pool = ctx.enter_context(tc.tile_pool(name="wt", bufs=1))
        xt_pool = ctx.enter_context(tc.tile_pool(name="xt", bufs=1))
        kvq_pool = ctx.enter_context(tc.tile_pool(name="kvq", bufs=2))
        sq_pool = ctx.enter_context(tc.tile_pool(name="sq", bufs=2))
        stat_pool = ctx.enter_context(tc.tile_pool(name="stat", bufs=2))
        qt_pool = ctx.enter_context(tc.tile_pool(name="qt", bufs=1))
        at_pool = ctx.enter_context(tc.tile_pool(name="at", bufs=1))
        bd_pool = ctx.enter_context(tc.tile_pool(name="bd", bufs=1))
        out_pool = ctx.enter_context(tc.tile_pool(name="osb", bufs=3))
        proj_psum = ctx.enter_context(
            tc.tile_pool(name="proj_psum", bufs=6, space="PSUM"))
        kv_sb_pool = ctx.enter_context(tc.tile_pool(name="kvacc", bufs=1))
        tp_psum = ctx.enter_context(
            tc.tile_pool(name="tp_psum", bufs=2, space="PSUM"))

        # ---- prologue: per-head v scale C^-sigmoid(norm_const) -> [128, H]
        svec = singles.tile([1, H], F32, name="svec")
        nc.sync.dma_start(out=svec[:], in_=ncst_ext[0, :, 0, 0])
        ssig = singles.tile([1, H], F32, name="ssig")
        nc.scalar.activation(ssig[:], svec[:],
                             mybir.ActivationFunctionType.Sigmoid)
        sexp = singles.tile([1, H], F32, name="sexp")
        nc.scalar.activation(sexp[:], ssig[:],
                             mybir.ActivationFunctionType.Exp, scale=-LN_C)

        ident = singles.tile([P, P], BF16, name="ident")
        make_identity(nc, ident[:])

        # ---- operand tiles (bf16).  x is ct-major so each host slice
        # lands as one contiguous per-partition run.
        wt = {
            w: wt_pool.tile([P, KC, D], BF16, name=f"wt_{w}", tag=f"wt_{w}")
            for w in ("Wk", "Wv", "Wq", "Wo")
        }
        xts = [
            xt_pool.tile([P, CT, KC, P], BF16, name=f"xt{n}", tag=f"xt{n}")
            for n in range(B)
        ]

        # ---- loads: one SWDGE cast DMA per chunk (f32 DRAM -> bf16
        # SBUF, contiguous both sides), emission order == priority.
        def load_w_half(wname, kc, half):
            ms = slice(half * 512, (half + 1) * 512)
            nc.gpsimd.dma_start(out=wt[wname][:, kc, ms],
                                in_=wT_ext[wname][:, kc, ms])

        def load_w_kc(wname, kc):
            nc.gpsimd.dma_start(out=wt[wname][:, kc, :],
                                in_=wT_ext[wname][:, kc, :])

        def load_x_slice(n, ct):
            nc.gpsimd.dma_start(out=xts[n][:, ct], in_=xT_ext[n, ct])

        load_x_slice(0, 0)
        for kc in range(KC):
            load_w_half("Wk", kc, 0)
        load_x_slice(0, 1)
        for kc in range(KC):
            load_w_half("Wk", kc, 1)
        for ct in range(2, CT):
            load_x_slice(0, ct)

        sv128 = singles.tile([P, H], F32, name="sv128")
        nc.gpsimd.partition_broadcast(sv128[:], sexp[0:1, :])

        # per-batch block-diagonal kv (bf16), memset early; diagonal
        # 64x64 blocks filled by DVE casts after kv accumulation
        bdall = [
            bd_pool.tile([P, D], BF16, name=f"bdall{n}", tag=f"bdall{n}")
            for n in range(B)
        ]
        for n in range(B):
            nc.gpsimd.memset(bdall[n][:], 0.0)

        for kc in range(KC):
            load_w_half("Wv", kc, 0)
        for kc in range(KC):
            load_w_half("Wv", kc, 1)
        for ct in range(CT):
            load_x_slice(1, ct)
        for kc in range(KC):
            load_w_kc("Wq", kc)
        for kc in range(KC):
            load_w_kc("Wo", kc)

        # ---- warm-up filler: dummy ident matmuls keep the PE busy while
        # the startup casts trickle in, so the HAM 8/8 clock upgrade
        # (needs ~3-4us of CONTINUOUS PE busy) engages early instead of
        # being reset by every sub-2us data-arrival gap.  They depend
        # only on ident (resident by ~9us) so they never stall the PE.
        dps = tp_psum.tile([P, 512], F32, name="dps", tag="pst")

        def dummy_mms(k):
            for _ in range(k):
                nc.tensor.matmul(dps[:, 0:P], ident[:], ident[:],
                                 start=True, stop=True)

        dummy_mms(20)

        # ---- phase helpers ------------------------------------------------
        def project(n, wname, ct, pname, ps=None, mcs=(0, 1), filler=0):
            if ps is None:
                ps = {}
            for mc in mcs:
                ps[mc] = proj_psum.tile([P, 512], F32,
                                        name=f"ps{pname}_{mc}", tag="proj")
                for kc in range(KC):
                    nc.tensor.matmul(
                        ps[mc][:],
                        xts[n][:, ct, kc, :],
                        wt[wname][:, kc, mc * 512:(mc + 1) * 512],
                        start=(kc == 0),
                        stop=(kc == KC - 1),
                    )
                    if filler and kc < KC - 1:
                        dummy_mms(filler)
            return ps

        def group_sumsq(ps, ssname):
            ss = stat_pool.tile([P, H], F32, name=ssname, tag=ssname)
            for mc in range(MC):
                sq = sq_pool.tile([P, 512], F32, name="sq", tag="sq")
                nc.scalar.square(sq[:], ps[mc][:])
                nc.vector.tensor_reduce(
                    ss[:, mc * 8:(mc + 1) * 8],
                    sq[:].rearrange("p (g d) -> p g d", g=8),
                    mybir.AxisListType.X,
                    mybir.AluOpType.add,
                )
            return ss

        def rsqrt_(ss, rname):
            r = stat_pool.tile([P, H], F32, name=rname, tag=rname)
            nc.vector.tensor_scalar_max(r[:], ss[:], 1e-30)
            nc.vector.reciprocal(r[:], r[:])
            nc.scalar.sqrt(r[:], r[:])
            return r

        def scaled_to_bf16(ps, r, outname, tag=None):
            o = kvq_pool.tile([P, D], BF16, name=outname,
                              tag=tag or outname, bufs=3)
            for mc in range(MC):
                ms = slice(mc * 512, (mc + 1) * 512)
                nc.vector.tensor_mul(
                    o[:, ms].rearrange("p (g d) -> p g d", g=8),
                    ps[mc][:].rearrange("p (g d) -> p g d", g=8),
                    r[:, mc * 8:(mc + 1) * 8][:, :, None]
                    .broadcast_to((P, 8, HD)),
                )
            return o

        def finish_K_tile(ct, ps, ksbs, ssks):
            ssks.append(group_sumsq(ps, f"ssk_{ct}"))
            ksb = kvq_pool.tile([P, D], BF16, name=f"ksb_{ct}",
                                tag=f"ksb_{ct}", bufs=1)
            for mc in range(MC):
                ms = slice(mc * 512, (mc + 1) * 512)
                nc.any.tensor_copy(ksb[:, ms], ps[mc][:])
            ksbs.append(ksb)

        def phase_K(n, warmup=False):
            # warmup: pad the first DMA-arrival-paced tiles with dummy
            # matmuls so the PE stays continuously busy
            fill = {0: 3, 1: 3, 2: 2, 3: 2} if warmup else {}
            ksbs, ssks = [], []
            for ct in range(CT):
                ps = project(n, "Wk", ct, "K", filler=fill.get(ct, 0))
                finish_K_tile(ct, ps, ksbs, ssks)
            return ksbs, ssks

        # ---- phase A-V: V projections + kv accumulation (SBUF f32).
        # kv matmuls for tile ct are emitted after tile ct+1's V matmuls;
        # the final tile's kv matmuls are deferred into phase Q.
        def make_kv_partial(ksbs, kvsb):
            def kv_partial(ct, vsb):
                for b in range(2):
                    kvp = proj_psum.tile([P, 512], F32, name=f"kvp_{b}",
                                         tag="proj")
                    for j in range(4):
                        hp = b * 4 + j
                        hs = slice(hp * P, (hp + 1) * P)
                        nc.tensor.matmul(
                            kvp[:, j * P:(j + 1) * P],
                            ksbs[ct][:, hs],
                            vsb[:, hs],
                            start=True,
                            stop=True,
                        )
                    if ct == 0:
                        nc.vector.tensor_copy(kvsb[b][:], kvp[:])
                    else:
                        nc.vector.tensor_add(kvsb[b][:], kvsb[b][:], kvp[:])
            return kv_partial

        def phase_V(n, ksbs, ssks, warmup=False):
            kvsb = [
                kv_sb_pool.tile([P, 512], F32, name=f"kvsb_{b}",
                                tag=f"kvsb_{b}")
                for b in range(2)
            ]
            kv_partial = make_kv_partial(ksbs, kvsb)
            fill = {0: 1, 1: 1} if warmup else {}
            prev = None
            for ct in range(CT):
                psV = project(n, "Wv", ct, "V", filler=fill.get(ct, 0))
                if prev is not None:
                    kv_partial(*prev)
                rk = rsqrt_(ssks[ct], "rk")
                rkv = stat_pool.tile([P, H], F32, name="rkv", tag="rkv")
                nc.vector.tensor_mul(rkv[:], rk[:], sv128[:])
                vsb = scaled_to_bf16(psV, rkv, "vsb")
                prev = (ct, vsb)
            return kvsb, kv_partial, prev

        # extract block-diagonal 64x64 blocks of kvsb into the pre-zeroed
        # bf16 tile (4 strided DVE casts)
        def bd_extract(n, kvsb):
            bdv = bdall[n][:].rearrange("p (h q) -> p h q", q=P)
            for b in range(2):
                srcv = kvsb[b][:].rearrange("p (j q) -> p j q", q=P)
                nc.vector.tensor_copy(
                    bdv[0:64, b * 4:(b + 1) * 4, 0:64],
                    srcv[0:64, :, 0:64])
                nc.vector.tensor_copy(
                    bdv[64:P, b * 4:(b + 1) * 4, 64:P],
                    srcv[64:P, :, 64:P])

        # ---- phase A-Q: Q projections + l2norm + PE transpose into q^T.
        # The deferred last kv_partial of phase V is emitted after the
        # second Q projection so its vsb scale has drained.
        def phase_Q(n, kv_tail):
            kvsb, kv_partial, prev = kv_tail
            qt = qt_pool.tile([P, KC, C], BF16, name="qt", tag="qt")

            def q_transpose(ct, qsb):
                cs = slice(ct * P, (ct + 1) * P)
                for g in range(2):
                    pst = tp_psum.tile([P, 512], BF16, name="pst", tag="pst")
                    for j in range(4):
                        mt = g * 4 + j
                        nc.tensor.transpose(pst[:, j * P:(j + 1) * P],
                                            qsb[:, mt * P:(mt + 1) * P],
                                            ident[:])
                    nc.any.tensor_copy(
                        qt[:, g * 4:(g + 1) * 4, cs],
                        pst[:].rearrange("p (j m) -> p j m", j=4))

            prevq = None
            for ct in range(CT):
                psQ = project(n, "Wq", ct, "Q")
                if ct == 1 and prev is not None:
                    kv_partial(*prev)
                    bd_extract(n, kvsb)
                    prev = None
                if prevq is not None:
                    q_transpose(*prevq)
                ssq = group_sumsq(psQ, "ssq")
                rq = rsqrt_(ssq, "rq")
                qsb = scaled_to_bf16(psQ, rq, "qsb")
                prevq = (ct, qsb)
            q_transpose(*prevq)
            return qt

        # ---- phase C: attn^T strips = blockdiag(kv) @ q^T
        def phase_C(n, qt):
            ats = []
            for hp in range(HP):
                at = at_pool.tile([P, C], BF16, name=f"at_{hp}",
                                  tag=f"at_{hp}")
                for cc in range(MC):
                    ccs = slice(cc * 512, (cc + 1) * 512)
                    psA = proj_psum.tile([P, 512], F32, name="psA",
                                         tag="proj")
                    nc.tensor.matmul(psA[:],
                                     bdall[n][:, hp * P:(hp + 1) * P],
                                     qt[:, hp, ccs],
                                     start=True, stop=True)
                    nc.any.tensor_copy(at[:, ccs], psA[:])
                ats.append(at)
            return ats

        # ---- phase D: out = attn^T.T @ Wo.T  (osb DMA on the scalar
        # HWDGE queue)
        def phase_D(n, ats, cts):
            for ct in cts:
                cs = slice(ct * P, (ct + 1) * P)
                psO = [
                    proj_psum.tile([P, 512], F32, name=f"psO_{mc}",
                                   tag="proj")
                    for mc in range(MC)
                ]
                for hp in range(HP):
                    for mc in range(MC):
                        nc.tensor.matmul(
                            psO[mc][:],
                            ats[hp][:, cs],
                            wt["Wo"][:, hp, mc * 512:(mc + 1) * 512],
                            start=(hp == 0),
                            stop=(hp == HP - 1),
                        )
                for mc in range(MC):
                    ms = slice(mc * 512, (mc + 1) * 512)
                    osb = out_pool.tile([P, 512], F32, name="osb", tag="osb")
                    nc.any.tensor_copy(osb[:], psO[mc][:])
                    nc.scalar.dma_start(out=out_ext[n, cs, ms], in_=osb[:])

        # ---- global schedule: batch 1's K phase fills batch 0's
        # C/D boundary; batch 0's last two D tiles fill batch 1's
        # Q->C boundary.
        ksbs0, ssks0 = phase_K(0, warmup=True)
        kv_tail0 = phase_V(0, ksbs0, ssks0, warmup=True)
        qt0 = phase_Q(0, kv_tail0)

        ksbs1, ssks1 = phase_K(1)

        ats0 = phase_C(0, qt0)
        phase_D(0, ats0, range(0, 6))

        kv_tail1 = phase_V(1, ksbs1, ssks1)
        qt1 = phase_Q(1, kv_tail1)

        phase_D(0, ats0, range(6, CT))

        ats1 = phase_C(1, qt1)
        phase_D(1, ats1, range(CT))

    nc.compile()
    return nc


_NC_CACHE = None


def _get_graph():
    global _NC_CACHE
    if _NC_CACHE is None:
        _NC_CACHE = build_graph()
    return _NC_CACHE


def _wT_blocked(W):
    # [P, KC, D] f32, wT[p, kc, m] = W[m, kc*128+p]
    # (pure reindexing of the f32 values)
    Wt = np.ascontiguousarray(W.T)                 # [k, m]
    Wt = Wt.reshape(KC, P, D)                      # [kc, p, m]
    return np.ascontiguousarray(Wt.transpose(1, 0, 2))


def _xT_blocked(xn):
    # [CT, P, KC, P] f32, xT[ct, p, kc, j] = x[ct*128+j, kc*128+p]
    xt = np.ascontiguousarray(xn.T)                # [k, c]
    xt = xt.reshape(KC, P, CT, P)                  # [kc, p, ct, j]
    return np.ascontiguousarray(xt.transpose(2, 1, 0, 3))


def kernel(x, Wq, Wk, Wv, Wo, norm_const, _trace=False):
    x = np.ascontiguousarray(np.asarray(x, dtype=np.float32))
    Wq = np.ascontiguousarray(np.asarray(Wq, dtype=np.float32))
    Wk = np.ascontiguousarray(np.asarray(Wk, dtype=np.float32))
    Wv = np.ascontiguousarray(np.asarray(Wv, dtype=np.float32))
    Wo = np.ascontiguousarray(np.asarray(Wo, dtype=np.float32))
    norm_const = np.ascontiguousarray(np.asarray(norm_const, dtype=np.float32))

    wT = {w: _wT_blocked(m)
          for w, m in (("Wq", Wq), ("Wk", Wk), ("Wv", Wv), ("Wo", Wo))}

    nc = _get_graph()
    in_maps = []
    for c in range(N_CORES):
        xTc = np.stack([_xT_blocked(x[c * B + n]) for n in range(B)])
        in_maps.append({
            "xT": xTc,
            "WqT": wT["Wq"], "WkT": wT["Wk"],
            "WvT": wT["Wv"], "WoT": wT["Wo"],
            "norm_const": norm_const,
        })
    res = run_bass_kernel_spmd(nc, in_maps, list(range(N_CORES)),
                               trace=_trace)
    out = np.concatenate([res.results[c]["out"] for c in range(N_CORES)],
                         axis=0)
    if _trace:
        kernel.last_exec_time_ns = res.exec_time_ns
        kernel.last_results = res
    return out
